# revision 11
# baseline (speedup 1.0000x reference)
import os
import sys

import numpy as np

sys.path.insert(0, "/opt/trn_rl_repo")

import concourse.bass as bass  # noqa: E402
import concourse.bacc as bacc  # noqa: E402
import concourse.mybir as mybir  # noqa: E402
import concourse.tile as tile  # noqa: E402
from concourse.bass_utils import run_bass_kernel_spmd  # noqa: E402

# Problem shapes (hardcoded; see nn_CoupledAttention).
B, S, NH, K = 64, 512, 128, 32
H2 = 2 * K     # 64: GRU input/hidden
G3 = 3 * H2    # 192
NCORES = 8
BL = B // NCORES  # 8 batches per core

F32 = mybir.dt.float32
AF = mybir.ActivationFunctionType

_CACHE = {}
LAST_RESULT = None


def _build_nc():
    nc = bacc.Bacc("TRN2", target_bir_lowering=False, debug=False,
                   enable_asserts=False, num_devices=NCORES)

    h_in = nc.dram_tensor("h_in", [BL, S, NH], F32, kind="ExternalInput").ap()
    mask_in = nc.dram_tensor("mask_in", [BL, S], F32, kind="ExternalInput").ap()
    gt_in = nc.dram_tensor("gt_in", [2 * K, NH, NH], F32, kind="ExternalInput").ap()
    u_in = nc.dram_tensor("u_in", [NH, 2], F32, kind="ExternalInput").ap()
    wihT_in = nc.dram_tensor("wihT_in", [H2, G3], F32, kind="ExternalInput").ap()
    whhT_in = nc.dram_tensor("whhT_in", [H2, G3], F32, kind="ExternalInput").ap()
    r0_in = nc.dram_tensor("r0_in", [H2, BL], F32, kind="ExternalInput").ap()
    v_in = nc.dram_tensor("v_in", [H2, BL * BL], F32, kind="ExternalInput").ap()
    id_in = nc.dram_tensor("id_in", [128, 128], F32, kind="ExternalInput").ap()
    r_out = nc.dram_tensor("r_out", [BL, S, H2], F32, kind="ExternalOutput").ap()
    rv_out = nc.dram_tensor("rv_out", [BL, S], F32, kind="ExternalOutput").ap()

    from contextlib import ExitStack

    with tile.TileContext(nc) as tc:
        with ExitStack() as ctx:
            const = ctx.enter_context(tc.tile_pool(name="const", bufs=1))
            big = ctx.enter_context(tc.tile_pool(name="big", bufs=1))
            work = ctx.enter_context(tc.tile_pool(name="work", bufs=4))
            gates = ctx.enter_context(tc.tile_pool(name="gates", bufs=3))
            epi = ctx.enter_context(tc.tile_pool(name="epi", bufs=3))

            # ---- constants to SBUF ----
            u_sb = const.tile([NH, 2], F32)
            nc.sync.dma_start(u_sb[:], u_in[:])
            wihT_sb = const.tile([H2, G3], F32)
            nc.sync.dma_start(wihT_sb[:], wihT_in[:])
            whhT_sb = const.tile([H2, G3], F32)
            nc.sync.dma_start(whhT_sb[:], whhT_in[:])
            state0_sb = const.tile([H2, BL], F32)
            nc.sync.dma_start(state0_sb[:], r0_in[:])
            v_sb = const.tile([H2, BL * BL], F32)
            nc.sync.dma_start(v_sb[:], v_in[:])
            id_sb = const.tile([128, 128], F32)
            nc.sync.dma_start(id_sb[:], id_in[:])
            mask_sb = const.tile([BL, S], F32)
            nc.sync.dma_start(mask_sb[:], mask_in[:])

            # ---- big SBUF tensors, [channel, b, s] layouts ----
            hT_sb = big.tile([NH, BL, S], F32)      # h transposed: [i, b, s]
            beta_sb = big.tile([H2, BL, S], F32)    # tanh(h @ w.T) transposed
            xn_sb = big.tile([H2, BL, S], F32)      # x-side n-gate preact
            rT_sb = big.tile([H2, BL, S], F32)      # GRU outputs (and state)
            rv_sb = big.tile([BL, S], F32)
            wT_sb = const.tile([NH, H2], F32)       # wT[i, c] = w[c, i]
            maskT_sb = const.tile([128, S // 128, BL], F32)  # [s%128, blk, b]

            NBLK = S // 128  # 4

            with ExitStack() as pctx:
                psum_w = pctx.enter_context(
                    tc.tile_pool(name="psum_w", bufs=1, space="PSUM"))
                psum_pro = pctx.enter_context(
                    tc.tile_pool(name="psum_pro", bufs=2, space="PSUM"))

                # ---- Phase W: wT[i, c] = sum_j G[c, i, j] * u[j] ----
                pw = psum_w.tile([NH, H2], F32)
                for c in range(2 * K):
                    g_tile = work.tile([NH, NH], F32, tag="gtile")
                    nc.sync.dma_start(g_tile[:], gt_in[c])
                    ucol = u_sb[:, 0:1] if c < K else u_sb[:, 1:2]
                    nc.tensor.matmul(pw[:, c:c + 1], g_tile[:], ucol,
                                     start=True, stop=True)
                nc.vector.tensor_copy(wT_sb[:], pw[:])

                # ---- mask transpose: (BL, S) -> [s%128, blk, b] ----
                for blk in range(NBLK):
                    pmt = psum_pro.tile([128, BL], F32, tag="p")
                    nc.tensor.transpose(
                        pmt[:], mask_sb[:, blk * 128:(blk + 1) * 128],
                        id_sb[0:BL, 0:BL])
                    nc.vector.tensor_copy(maskT_sb[:, blk, :], pmt[:])

                # ---- Phase H: load h tiles, mask, transpose into hT ----
                for b in range(BL):
                    for blk in range(NBLK):
                        h_tile = work.tile([128, NH], F32, tag="htile")
                        nc.sync.dma_start(
                            h_tile[:], h_in[b, blk * 128:(blk + 1) * 128, :])
                        hm_tile = work.tile([128, NH], F32, tag="hmtile")
                        nc.vector.tensor_scalar_mul(
                            hm_tile[:], h_tile[:], maskT_sb[:, blk, b:b + 1])
                        pht = psum_pro.tile([NH, 128], F32, tag="p")
                        nc.tensor.transpose(pht[:], hm_tile[:], id_sb[:])
                        nc.scalar.copy(
                            hT_sb[:, b, blk * 128:(blk + 1) * 128], pht[:])

                # ---- Phase BETA: beta = tanh(wT.T @ hT), per batch ----
                for b in range(BL):
                    pb = psum_pro.tile([H2, S], F32, tag="p")
                    nc.tensor.matmul(pb[:], wT_sb[:], hT_sb[:, b, :],
                                     start=True, stop=True)
                    nc.scalar.activation(beta_sb[:, b, :], pb[:], AF.Tanh)

                # ---- Phase XN: xn = Wih_n.T.T @ beta, per batch ----
                for b in range(BL):
                    px = psum_pro.tile([H2, S], F32, tag="p")
                    nc.tensor.matmul(px[:], wihT_sb[:, 128:192],
                                     beta_sb[:, b, :], start=True, stop=True)
                    nc.vector.tensor_copy(xn_sb[:, b, :], px[:])

            # ---- GRU + interleaved epilogue ----
            psum_gru = ctx.enter_context(
                tc.tile_pool(name="psum_gru", bufs=4, space="PSUM"))
            psum_epi = ctx.enter_context(
                tc.tile_pool(name="psum_epi", bufs=1, space="PSUM"))

            wih_r = wihT_sb[:, 0:64]
            wih_z = wihT_sb[:, 64:128]
            whh_r = whhT_sb[:, 0:64]
            whh_z = whhT_sb[:, 64:128]
            whh_n = whhT_sb[:, 128:192]

            NSTEP = int(os.environ.get("KERNEL_GRU_STEPS", S))
            for s in range(NSTEP):
                state = state0_sb[:] if s == 0 else rT_sb[:, :, s - 1]
                x_s = beta_sb[:, :, s]

                pr = psum_gru.tile([H2, BL], F32, tag="g")
                nc.tensor.matmul(pr[:], wih_r, x_s, start=True, stop=False)
                nc.tensor.matmul(pr[:], whh_r, state, start=False, stop=True)
                pz = psum_gru.tile([H2, BL], F32, tag="g")
                nc.tensor.matmul(pz[:], wih_z, x_s, start=True, stop=False)
                nc.tensor.matmul(pz[:], whh_z, state, start=False, stop=True)
                pn = psum_gru.tile([H2, BL], F32, tag="g")
                nc.tensor.matmul(pn[:], whh_n, state, start=True, stop=True)

                rg = gates.tile([H2, BL], F32, tag="rg")
                nc.scalar.activation(rg[:], pr[:], AF.Sigmoid)
                zg = gates.tile([H2, BL], F32, tag="zg")
                nc.scalar.activation(zg[:], pz[:], AF.Sigmoid)

                t1 = gates.tile([H2, BL], F32, tag="t1")
                nc.vector.tensor_mul(t1[:], rg[:], pn[:])
                t2 = gates.tile([H2, BL], F32, tag="t2")
                nc.vector.tensor_add(t2[:], t1[:], xn_sb[:, :, s])
                ng = gates.tile([H2, BL], F32, tag="ng")
                nc.scalar.activation(ng[:], t2[:], AF.Tanh)

                d = gates.tile([H2, BL], F32, tag="d")
                nc.vector.tensor_sub(d[:], state, ng[:])
                e = gates.tile([H2, BL], F32, tag="e")
                nc.vector.tensor_mul(e[:], d[:], zg[:])
                nc.vector.tensor_add(rT_sb[:, :, s], e[:], ng[:])

                # epilogue for each finished 128-step block
                if s % 128 == 127:
                    blk = s // 128
                    sl = slice(blk * 128, (blk + 1) * 128)
                    for b in range(BL):
                        prt = psum_epi.tile([128, H2], F32, tag="pe")
                        nc.tensor.transpose(
                            prt[:], rT_sb[:, b, blk * 128:(blk + 1) * 128],
                            id_sb[0:H2, 0:H2])
                        rnat = epi.tile([128, H2], F32, tag="rnat")
                        nc.scalar.mul(rnat[:], prt[:],
                                      maskT_sb[:, blk, b:b + 1])
                        nc.sync.dma_start(
                            r_out[b, blk * 128:(blk + 1) * 128, :], rnat[:])

                    prv = psum_epi.tile([BL, 128], F32, tag="pe")
                    for b in range(BL):
                        nc.tensor.matmul(
                            prv[:], v_sb[:, b * BL:(b + 1) * BL],
                            rT_sb[:, b, blk * 128:(blk + 1) * 128],
                            start=(b == 0), stop=(b == BL - 1))
                    nc.vector.tensor_mul(
                        rv_sb[:, sl], prv[:], mask_sb[:, sl])

            if NSTEP >= 128:
                nc.sync.dma_start(rv_out[:], rv_sb[:])

    nc.compile()
    return nc


def kernel(h, u_a, u_o, mask, G_a, G_o, r0, v, W_ih, W_hh, trace=None):
    global LAST_RESULT
    h = np.asarray(h, np.float32)
    u_a = np.asarray(u_a, np.float32)
    u_o = np.asarray(u_o, np.float32)
    mask_f = np.asarray(mask).astype(np.float32)
    G_a = np.asarray(G_a, np.float32)
    G_o = np.asarray(G_o, np.float32)
    r0 = np.asarray(r0, np.float32)
    v = np.asarray(v, np.float32)
    W_ih = np.asarray(W_ih, np.float32)
    W_hh = np.asarray(W_hh, np.float32)

    if "nc" not in _CACHE:
        _CACHE["nc"] = _build_nc()
    nc = _CACHE["nc"]

    gt = np.concatenate([
        np.ascontiguousarray(G_a[:, 0].transpose(0, 2, 1)),
        np.ascontiguousarray(G_o[:, 0].transpose(0, 2, 1)),
    ], axis=0)  # (2K, j, i)
    u2 = np.ascontiguousarray(np.concatenate([u_a, u_o], axis=1))  # (NH, 2)
    wihT = np.ascontiguousarray(W_ih.T)
    whhT = np.ascontiguousarray(W_hh.T)
    r0r = np.ascontiguousarray(np.broadcast_to(r0.reshape(H2, 1), (H2, BL)))
    vc = np.zeros((H2, BL * BL), np.float32)
    for b in range(BL):
        vc[:, b * BL + b] = v
    id128 = np.eye(128, dtype=np.float32)

    in_maps = []
    for c in range(NCORES):
        in_maps.append({
            "h_in": np.ascontiguousarray(h[c * BL:(c + 1) * BL]),
            "mask_in": np.ascontiguousarray(mask_f[c * BL:(c + 1) * BL]),
            "gt_in": gt,
            "u_in": u2,
            "wihT_in": wihT,
            "whhT_in": whhT,
            "r0_in": r0r,
            "v_in": vc,
            "id_in": id128,
        })

    if trace is None:
        trace = bool(int(os.environ.get("KERNEL_TRACE", "0")))
    res = run_bass_kernel_spmd(nc, in_maps, list(range(NCORES)), trace=trace)
    LAST_RESULT = res

    r = np.concatenate([res.results[c]["r_out"] for c in range(NCORES)], axis=0)
    rv = np.concatenate([res.results[c]["rv_out"] for c in range(NCORES)], axis=0)
    return r, rv
